# revision 4
# baseline (speedup 1.0000x reference)
"""Darknet 3x3 conv block (conv * mask + bias) on 8 TRN2 NeuronCores.

Problem: x[1,512,192,192] (*) w[512,512,3,3] stride1 pad1, then *mask + bias.

Strategy:
  - Spatial shard over H: each core computes 24 output rows x all 512 F.
  - Host packs: x zero-padded to [512,194,194], per-core slice of 26 rows,
    channel-chunked to [128, 4cc, 26, 194] bf16.  Weights transposed to
    [c_local, cc, tap, F] bf16 (tap = kh*3+kw).  Mask replicated across
    partitions as [128, 12, 384] f32.  Bias as [128, 4fm] f32.
  - Device: implicit GEMM.  Output tile [F=128, px=384] (= 2 rows x 192
    cols) accumulates 36 matmuls in PSUM (4 C-chunks x 9 taps); lhsT =
    w tile [c128, F128], rhs = shifted x window [c128, 2x192] (2D AP on
    the padded x slab - no im2col materialization).
  - Epilogue: DVE multiply by mask (PSUM read), ScalarE bias-add, DMA out.
  - Host unshard: concat 8 row-slabs, reshape to [1,512,192,192] f32.
"""

import sys

for _p in ("/opt/trn_rl_repo",):
    if _p not in sys.path:
        sys.path.insert(0, _p)

import numpy as np
import ml_dtypes

N_CORES = 8
C = 512
F = 512
H = 192
W = 192
HC = H // N_CORES          # output rows per core = 24
RP = HC // 2               # row-pairs per core = 12
PX = 2 * W                 # px per output tile = 384
CC = C // 128              # c chunks = 4
FM = F // 128              # f chunks = 4
TAPS = 9

_CACHE = {}


def _build():
    import concourse.bacc as bacc
    import concourse.mybir as mybir
    from concourse.tile import TileContext

    BF = mybir.dt.bfloat16
    F32 = mybir.dt.float32

    nc = bacc.Bacc(trn_type="TRN2", num_devices=N_CORES)
    x_sh = nc.dram_tensor("x_sh", [128, CC, HC + 2, W + 2], BF, kind="ExternalInput")
    w_sh = nc.dram_tensor("w_sh", [128, CC, TAPS, F], BF, kind="ExternalInput")
    maskb = nc.dram_tensor("maskb", [128, RP, PX], F32, kind="ExternalInput")
    b_sh = nc.dram_tensor("b_sh", [128, FM], F32, kind="ExternalInput")
    y_sh = nc.dram_tensor("y_sh", [FM, 128, RP, PX], F32, kind="ExternalOutput")

    with TileContext(nc) as tc:
        with (
            tc.tile_pool(name="const", bufs=1) as cpool,
            tc.tile_pool(name="psum", bufs=8, space="PSUM") as ppool,
            tc.tile_pool(name="outp", bufs=6) as opool,
        ):
            xt = cpool.tile([128, CC, HC + 2, W + 2], BF)
            wt = cpool.tile([128, CC, TAPS, F], BF)
            for cc in range(CC):
                nc.sync.dma_start(out=xt[:, cc], in_=x_sh[:, cc])
                nc.sync.dma_start(out=wt[:, cc], in_=w_sh[:, cc])
            mt = cpool.tile([128, RP, PX], F32)
            nc.sync.dma_start(out=mt[:], in_=maskb[:])
            bt = cpool.tile([128, FM], F32)
            nc.sync.dma_start(out=bt[:], in_=b_sh[:])

            GRP = 4  # px tiles accumulated concurrently (PSUM banks)
            for fm in range(FM):
                for g in range(RP // GRP):
                    psums = [
                        ppool.tile([128, PX], F32, name=f"ps_{fm}_{g}_{j}", tag="ps")
                        for j in range(GRP)
                    ]
                    for a in range(CC * TAPS):
                        cc, o = divmod(a, TAPS)
                        kh, kw = divmod(o, 3)
                        lhsT = wt[:, cc, o, fm * 128:(fm + 1) * 128]
                        for j in range(GRP):
                            t = g * GRP + j
                            rhs = xt[:, cc, 2 * t + kh:2 * t + kh + 2, kw:kw + W]
                            nc.tensor.matmul(
                                psums[j], lhsT, rhs,
                                start=(a == 0), stop=(a == CC * TAPS - 1),
                            )
                    for j in range(GRP):
                        t = g * GRP + j
                        ot = opool.tile([128, PX], F32, name=f"ot_{fm}_{g}_{j}", tag="ot")
                        nc.vector.tensor_mul(ot[:], psums[j][:], mt[:, t])
                        nc.scalar.activation(
                            ot[:], ot[:],
                            mybir.ActivationFunctionType.Identity,
                            bias=bt[:, fm:fm + 1],
                        )
                        nc.sync.dma_start(out=y_sh[fm, :, t], in_=ot[:])

    nc.compile()
    return nc


def _pack(x, w, b, mask):
    x = np.asarray(x, dtype=np.float32)
    w = np.asarray(w, dtype=np.float32)
    b = np.asarray(b, dtype=np.float32)
    mask = np.asarray(mask)

    xp = np.zeros((C, H + 2, W + 2), dtype=np.float32)
    xp[:, 1:-1, 1:-1] = x[0]
    xp = xp.astype(ml_dtypes.bfloat16)

    # [kh,kw,c,f] -> [tap, cc, c_local, f] -> [c_local, cc, tap, f]
    wt = w.transpose(2, 3, 1, 0).reshape(TAPS, CC, 128, F)
    wt = np.ascontiguousarray(wt.transpose(2, 1, 0, 3)).astype(ml_dtypes.bfloat16)

    b_re = np.ascontiguousarray(b.reshape(FM, 128).T)

    mf = mask.astype(np.float32)
    in_maps = []
    for k in range(N_CORES):
        xs = xp[:, HC * k:HC * k + HC + 2, :]                 # [512, 26, 194]
        xs = np.ascontiguousarray(
            xs.reshape(CC, 128, HC + 2, W + 2).transpose(1, 0, 2, 3))
        ms = mf[HC * k:HC * k + HC].reshape(RP, PX)
        ms = np.ascontiguousarray(np.broadcast_to(ms, (128, RP, PX)))
        in_maps.append({"x_sh": xs, "w_sh": wt, "maskb": ms, "b_sh": b_re})
    return in_maps


def _unpack(results):
    slabs = []
    for k in range(N_CORES):
        ys = results[k]["y_sh"]                               # [4, 128, 12, 384]
        slabs.append(ys.reshape(F, HC, W))
    out = np.concatenate(slabs, axis=1)                       # [512, 192, 192]
    return out[None].astype(np.float32)


def _run(inputs, **run_kwargs):
    from concourse.bass_utils import run_bass_kernel_spmd

    if "nc" not in _CACHE:
        _CACHE["nc"] = _build()
    nc = _CACHE["nc"]
    in_maps = _pack(inputs["x"], inputs["w"], inputs["b"], inputs["mask"])
    res = run_bass_kernel_spmd(nc, in_maps, core_ids=list(range(N_CORES)), **run_kwargs)
    return _unpack(res.results), res


def kernel(**inputs):
    out, _ = _run(inputs)
    return out


# revision 5
# speedup vs baseline: 1.0071x; 1.0071x over previous
"""Darknet 3x3 conv block (conv * mask + bias) on 8 TRN2 NeuronCores.

Problem: x[1,512,192,192] (*) w[512,512,3,3] stride1 pad1, then *mask + bias.

Strategy:
  - Spatial shard over H: each core computes 24 output rows x all 512 F.
  - Host packs: x zero-padded to [512,194,194], per-core slice of 26 rows,
    channel-chunked to [128, 4cc, 26, 194] bf16.  Weights transposed to
    [c_local, fm, cc, tap, f128] bf16 (tap = kh*3+kw).  Mask replicated
    across partitions as [128, 12, 384] f32.  Bias as [128, 4fm] f32.
  - Device: implicit GEMM.  Output tile [F=128, px=384] (= 2 rows x 192
    cols) accumulates 36 matmuls in PSUM (4 C-chunks x 9 taps); lhsT =
    w tile [c128, F128], rhs = shifted x window [c128, 2x192] (2D AP on
    the padded x slab - no im2col materialization).
  - Epilogue: DVE multiply by mask (PSUM read), ScalarE bias-add, DMA out.
  - Head hiding: ~48 warmup matmuls on a memset scratch tile keep the PE
    busy (and HAM-warm) while the first input DMAs land; DMAs are split
    and ordered by first use across both HWDGE rings (x on SP, w on ACT).
  - Host unshard: concat 8 row-slabs, reshape to [1,512,192,192] f32.
"""

import sys

for _p in ("/opt/trn_rl_repo",):
    if _p not in sys.path:
        sys.path.insert(0, _p)

import numpy as np
import ml_dtypes

N_CORES = 8
C = 512
F = 512
H = 192
W = 192
HC = H // N_CORES          # output rows per core = 24
RP = HC // 2               # row-pairs per core = 12
PX = 2 * W                 # px per output tile = 384
CC = C // 128              # c chunks = 4
FM = F // 128              # f chunks = 4
TAPS = 9
NWARM = 48                 # PE warmup matmuls (~10us at N=512)

_CACHE = {}


def _build():
    import concourse.bacc as bacc
    import concourse.mybir as mybir
    from concourse.tile import TileContext

    BF = mybir.dt.bfloat16
    F32 = mybir.dt.float32

    nc = bacc.Bacc(trn_type="TRN2", num_devices=N_CORES)
    x_sh = nc.dram_tensor("x_sh", [128, CC, HC + 2, W + 2], BF, kind="ExternalInput")
    w_sh = nc.dram_tensor("w_sh", [128, FM, CC, TAPS, 128], BF, kind="ExternalInput")
    maskb = nc.dram_tensor("maskb", [128, RP, PX], F32, kind="ExternalInput")
    b_sh = nc.dram_tensor("b_sh", [128, FM], F32, kind="ExternalInput")
    y_sh = nc.dram_tensor("y_sh", [FM, 128, RP, PX], F32, kind="ExternalOutput")

    # x row-thirds, in first-use order: rows 0-9 cover group g=0 (+halo),
    # 10-17 cover g=1, 18-25 cover g=2.
    ROW_SPLITS = [(0, 10), (10, 8), (18, 8)]

    with TileContext(nc) as tc:
        with (
            tc.tile_pool(name="const", bufs=1) as cpool,
            tc.tile_pool(name="psum", bufs=8, space="PSUM") as ppool,
            tc.tile_pool(name="outp", bufs=6) as opool,
        ):
            # PE warmup: matmuls on a zeroed scratch tile, no DMA deps.
            scratch = cpool.tile([128, 512], BF)
            nc.gpsimd.memset(scratch[:], 0.0)
            dps = ppool.tile([128, PX], F32, name="dps", tag="ps")
            for _ in range(NWARM):
                nc.tensor.matmul(dps[:, :PX], scratch[:, :128], scratch[:, :PX],
                                 start=True, stop=True)

            xt = cpool.tile([128, CC, HC + 2, W + 2], BF)
            wt = cpool.tile([128, FM, CC, TAPS, 128], BF)
            # First-use ordered loads.  x rides the SP HWDGE ring, w/mask/b
            # ride the ACT ring, so the two streams run in parallel.
            nc.scalar.dma_start(out=wt[:, 0], in_=w_sh[:, 0])
            for r0, nr in ROW_SPLITS:
                for cc in range(CC):
                    nc.sync.dma_start(out=xt[:, cc, r0:r0 + nr],
                                      in_=x_sh[:, cc, r0:r0 + nr])
            mt = cpool.tile([128, RP, PX], F32)
            nc.scalar.dma_start(out=mt[:], in_=maskb[:])
            bt = cpool.tile([128, FM], F32)
            nc.scalar.dma_start(out=bt[:], in_=b_sh[:])
            for fm in range(1, FM):
                nc.scalar.dma_start(out=wt[:, fm], in_=w_sh[:, fm])

            GRP = 4  # px tiles accumulated concurrently (PSUM banks)
            for fm in range(FM):
                for g in range(RP // GRP):
                    psums = [
                        ppool.tile([128, PX], F32, name=f"ps_{fm}_{g}_{j}", tag="ps")
                        for j in range(GRP)
                    ]
                    for a in range(CC * TAPS):
                        cc, o = divmod(a, TAPS)
                        kh, kw = divmod(o, 3)
                        lhsT = wt[:, fm, cc, o]
                        for j in range(GRP):
                            t = g * GRP + j
                            rhs = xt[:, cc, 2 * t + kh:2 * t + kh + 2, kw:kw + W]
                            nc.tensor.matmul(
                                psums[j], lhsT, rhs,
                                start=(a == 0), stop=(a == CC * TAPS - 1),
                            )
                    for j in range(GRP):
                        t = g * GRP + j
                        ot = opool.tile([128, PX], F32, name=f"ot_{fm}_{g}_{j}", tag="ot")
                        nc.vector.tensor_mul(ot[:], psums[j][:], mt[:, t])
                        nc.scalar.activation(
                            ot[:], ot[:],
                            mybir.ActivationFunctionType.Identity,
                            bias=bt[:, fm:fm + 1],
                        )
                        nc.sync.dma_start(out=y_sh[fm, :, t], in_=ot[:])

    nc.compile()
    return nc


def _pack(x, w, b, mask):
    x = np.asarray(x, dtype=np.float32)
    w = np.asarray(w, dtype=np.float32)
    b = np.asarray(b, dtype=np.float32)
    mask = np.asarray(mask)

    xp = np.zeros((C, H + 2, W + 2), dtype=np.float32)
    xp[:, 1:-1, 1:-1] = x[0]
    xp = xp.astype(ml_dtypes.bfloat16)

    # [kh,kw,c,f] -> [tap, cc, c_local, fm, f128] -> [c_local, fm, cc, tap, f128]
    wt = w.transpose(2, 3, 1, 0).reshape(TAPS, CC, 128, FM, 128)
    wt = np.ascontiguousarray(wt.transpose(2, 3, 1, 0, 4)).astype(ml_dtypes.bfloat16)

    b_re = np.ascontiguousarray(b.reshape(FM, 128).T)

    mf = mask.astype(np.float32)
    in_maps = []
    for k in range(N_CORES):
        xs = xp[:, HC * k:HC * k + HC + 2, :]                 # [512, 26, 194]
        xs = np.ascontiguousarray(
            xs.reshape(CC, 128, HC + 2, W + 2).transpose(1, 0, 2, 3))
        ms = mf[HC * k:HC * k + HC].reshape(RP, PX)
        ms = np.ascontiguousarray(np.broadcast_to(ms, (128, RP, PX)))
        in_maps.append({"x_sh": xs, "w_sh": wt, "maskb": ms, "b_sh": b_re})
    return in_maps


def _unpack(results):
    slabs = []
    for k in range(N_CORES):
        ys = results[k]["y_sh"]                               # [4, 128, 12, 384]
        slabs.append(ys.reshape(F, HC, W))
    out = np.concatenate(slabs, axis=1)                       # [512, 192, 192]
    return out[None].astype(np.float32)


def _run(inputs, **run_kwargs):
    from concourse.bass_utils import run_bass_kernel_spmd

    if "nc" not in _CACHE:
        _CACHE["nc"] = _build()
    nc = _CACHE["nc"]
    in_maps = _pack(inputs["x"], inputs["w"], inputs["b"], inputs["mask"])
    res = run_bass_kernel_spmd(nc, in_maps, core_ids=list(range(N_CORES)), **run_kwargs)
    return _unpack(res.results), res


def kernel(**inputs):
    out, _ = _run(inputs)
    return out


# revision 7
# speedup vs baseline: 1.0143x; 1.0072x over previous
"""Darknet 3x3 conv block (conv * mask + bias) on 8 TRN2 NeuronCores.

Problem: x[1,512,192,192] (*) w[512,512,3,3] stride1 pad1, then *mask + bias.

Strategy:
  - Spatial shard over H: each core computes 24 output rows x all 512 F.
  - Host packs: x zero-padded to [512,194,194], per-core slice of 26 rows,
    channel-chunked to [128, 4cc, 26, 194] bf16.  Weights transposed to
    [c_local, fm, cc, tap, f128] bf16 (tap = kh*3+kw).  Mask replicated
    across partitions as [128, 12, 384] f32.  Bias as [128, 4fm] f32.
  - Device: implicit GEMM.  Output tile [F=128, px=384] (= 2 rows x 192
    cols) accumulates 36 matmuls in PSUM (4 C-chunks x 9 taps); lhsT =
    w tile [c128, F128], rhs = shifted x window [c128, 2x192] (2D AP on
    the padded x slab - no im2col materialization).  Groups of 4 px
    tiles share one 4-bank PSUM tile -> one DVE mask-multiply, one
    ScalarE bias-add and one 768KB output DMA per group.
  - Head hiding: a few warmup matmuls on a scratch tile keep the PE busy
    (and HAM-warm) while the first input DMAs land; DMAs are split and
    ordered by first use across both HWDGE rings (x on SP, w on ACT).
  - Host unshard: concat 8 row-slabs, reshape to [1,512,192,192] f32.
"""

import sys

for _p in ("/opt/trn_rl_repo",):
    if _p not in sys.path:
        sys.path.insert(0, _p)

import numpy as np
import ml_dtypes

N_CORES = 8
C = 512
F = 512
H = 192
W = 192
HC = H // N_CORES          # output rows per core = 24
RP = HC // 2               # row-pairs per core = 12
PX = 2 * W                 # px per output tile = 384
CC = C // 128              # c chunks = 4
FM = F // 128              # f chunks = 4
TAPS = 9
NWARM = 14                 # PE warmup matmuls (~2.3us)
GRP = 4                    # px tiles per PSUM group (4 banks)

_CACHE = {}


def _build():
    import concourse.bacc as bacc
    import concourse.mybir as mybir
    from concourse.tile import TileContext

    BF = mybir.dt.bfloat16
    F32 = mybir.dt.float32

    nc = bacc.Bacc(trn_type="TRN2", num_devices=N_CORES)
    x_sh = nc.dram_tensor("x_sh", [128, CC, HC + 2, W + 2], BF, kind="ExternalInput")
    w_sh = nc.dram_tensor("w_sh", [128, FM, CC, TAPS, 128], BF, kind="ExternalInput")
    maskb = nc.dram_tensor("maskb", [128, RP, PX], F32, kind="ExternalInput")
    b_sh = nc.dram_tensor("b_sh", [128, FM], F32, kind="ExternalInput")
    y_sh = nc.dram_tensor("y_sh", [FM, 128, RP, PX], F32, kind="ExternalOutput")

    # x row-thirds, in first-use order: rows 0-9 cover group g=0 (+halo),
    # 10-17 cover g=1, 18-25 cover g=2.
    ROW_SPLITS = [(0, 10), (10, 8), (18, 8)]
    NG = RP // GRP

    with TileContext(nc) as tc:
        with (
            tc.tile_pool(name="const", bufs=1) as cpool,
            tc.tile_pool(name="psum", bufs=2, space="PSUM") as ppool,
            tc.tile_pool(name="outp", bufs=3) as opool,
        ):
            # PE warmup on a zeroed scratch tile (output is discarded; the
            # PSUM slot is overwritten by the first real start=True group).
            scratch = cpool.tile([128, PX], BF)
            nc.vector.memset(scratch[:], 0.0)
            dps = ppool.tile([128, 512], F32, name="dps", tag="ps")
            for _ in range(NWARM):
                nc.tensor.matmul(dps[:, :PX], scratch[:, :128], scratch[:],
                                 start=True, stop=True)

            xt = cpool.tile([128, CC, HC + 2, W + 2], BF)
            wt = cpool.tile([128, FM, CC, TAPS, 128], BF)
            # First-use ordered loads.  x rides the SP HWDGE ring, w/mask/b
            # ride the ACT ring, so the two streams run in parallel.
            r0, nr = ROW_SPLITS[0]
            for cc in range(CC):
                nc.scalar.dma_start(out=wt[:, 0, cc], in_=w_sh[:, 0, cc])
                nc.sync.dma_start(out=xt[:, cc, r0:r0 + nr],
                                  in_=x_sh[:, cc, r0:r0 + nr])
            for r0, nr in ROW_SPLITS[1:]:
                for cc in range(CC):
                    nc.sync.dma_start(out=xt[:, cc, r0:r0 + nr],
                                      in_=x_sh[:, cc, r0:r0 + nr])
            mt = cpool.tile([128, RP, PX], F32)
            nc.scalar.dma_start(out=mt[:], in_=maskb[:])
            bt = cpool.tile([128, FM], F32)
            nc.scalar.dma_start(out=bt[:], in_=b_sh[:])
            for fm in range(1, FM):
                nc.scalar.dma_start(out=wt[:, fm], in_=w_sh[:, fm])

            for fm in range(FM):
                for g in range(NG):
                    # one 4-bank PSUM tile holds the group's 4 output tiles
                    pt = ppool.tile([128, GRP, 512], F32, name=f"ps_{fm}_{g}",
                                    tag="ps")
                    for a in range(CC * TAPS):
                        cc, o = divmod(a, TAPS)
                        kh, kw = divmod(o, 3)
                        lhsT = wt[:, fm, cc, o]
                        for j in range(GRP):
                            t = g * GRP + j
                            rhs = xt[:, cc, 2 * t + kh:2 * t + kh + 2, kw:kw + W]
                            nc.tensor.matmul(
                                pt[:, j, :PX], lhsT, rhs,
                                start=(a == 0), stop=(a == CC * TAPS - 1),
                            )
                    last = (fm == FM - 1 and g == NG - 1)
                    if not last:
                        ot = opool.tile([128, GRP, PX], F32, name=f"ot_{fm}_{g}",
                                        tag="ot")
                        nc.vector.tensor_mul(ot[:], pt[:, :, :PX],
                                             mt[:, g * GRP:(g + 1) * GRP])
                        nc.scalar.activation(
                            ot[:], ot[:],
                            mybir.ActivationFunctionType.Identity,
                            bias=bt[:, fm:fm + 1],
                        )
                        nc.sync.dma_start(out=y_sh[fm, :, g * GRP:(g + 1) * GRP],
                                          in_=ot[:])
                    else:
                        # per-tile epilogue on the final group: keeps the
                        # exposed post-matmul chain short
                        for j in range(GRP):
                            t = g * GRP + j
                            otj = opool.tile([128, PX], F32, name=f"otl_{j}",
                                             tag="otl")
                            nc.vector.tensor_mul(otj[:], pt[:, j, :PX], mt[:, t])
                            nc.scalar.activation(
                                otj[:], otj[:],
                                mybir.ActivationFunctionType.Identity,
                                bias=bt[:, fm:fm + 1],
                            )
                            nc.sync.dma_start(out=y_sh[fm, :, t], in_=otj[:])

    nc.compile()
    return nc


def _pack(x, w, b, mask):
    x = np.asarray(x, dtype=np.float32)
    w = np.asarray(w, dtype=np.float32)
    b = np.asarray(b, dtype=np.float32)
    mask = np.asarray(mask)

    xp = np.zeros((C, H + 2, W + 2), dtype=np.float32)
    xp[:, 1:-1, 1:-1] = x[0]
    xp = xp.astype(ml_dtypes.bfloat16)

    # [kh,kw,c,f] -> [tap, cc, c_local, fm, f128] -> [c_local, fm, cc, tap, f128]
    wt = w.transpose(2, 3, 1, 0).reshape(TAPS, CC, 128, FM, 128)
    wt = np.ascontiguousarray(wt.transpose(2, 3, 1, 0, 4)).astype(ml_dtypes.bfloat16)

    b_re = np.ascontiguousarray(b.reshape(FM, 128).T)

    mf = mask.astype(np.float32)
    in_maps = []
    for k in range(N_CORES):
        xs = xp[:, HC * k:HC * k + HC + 2, :]                 # [512, 26, 194]
        xs = np.ascontiguousarray(
            xs.reshape(CC, 128, HC + 2, W + 2).transpose(1, 0, 2, 3))
        ms = mf[HC * k:HC * k + HC].reshape(RP, PX)
        ms = np.ascontiguousarray(np.broadcast_to(ms, (128, RP, PX)))
        in_maps.append({"x_sh": xs, "w_sh": wt, "maskb": ms, "b_sh": b_re})
    return in_maps


def _unpack(results):
    slabs = []
    for k in range(N_CORES):
        ys = results[k]["y_sh"]                               # [4, 128, 12, 384]
        slabs.append(ys.reshape(F, HC, W))
    out = np.concatenate(slabs, axis=1)                       # [512, 192, 192]
    return out[None].astype(np.float32)


def _run(inputs, **run_kwargs):
    from concourse.bass_utils import run_bass_kernel_spmd

    if "nc" not in _CACHE:
        _CACHE["nc"] = _build()
    nc = _CACHE["nc"]
    in_maps = _pack(inputs["x"], inputs["w"], inputs["b"], inputs["mask"])
    res = run_bass_kernel_spmd(nc, in_maps, core_ids=list(range(N_CORES)), **run_kwargs)
    return _unpack(res.results), res


def kernel(**inputs):
    out, _ = _run(inputs)
    return out


# revision 8
# speedup vs baseline: 1.0147x; 1.0004x over previous
"""Darknet 3x3 conv block (conv * mask + bias) on 8 TRN2 NeuronCores.

Problem: x[1,512,192,192] (*) w[512,512,3,3] stride1 pad1, then *mask + bias.

Strategy:
  - Spatial shard over H: each core computes 24 output rows x all 512 F.
  - Host packs: x zero-padded to [512,194,194], per-core slice of 26 rows,
    channel-chunked to [128, 4cc, 26, 194] bf16.  Weights transposed to
    [c_local, fm, cc, tap, f128] bf16 (tap = kh*3+kw).  Mask replicated
    across partitions as [128, 12, 384] f32.  Bias as [128, 4fm] f32.
  - Device: implicit GEMM.  Output tile [F=128, px=384] (= 2 rows x 192
    cols) accumulates 36 matmuls in PSUM (4 C-chunks x 9 taps); lhsT =
    w tile [c128, F128], rhs = shifted x window [c128, 2x192] (2D AP on
    the padded x slab - no im2col materialization).  Groups of 4 px
    tiles share one 4-bank PSUM tile -> one DVE mask-multiply, one
    ScalarE bias-add and one 768KB output DMA per group.
  - Head hiding: a few warmup matmuls on a scratch tile keep the PE busy
    (and HAM-warm) while the first input DMAs land; DMAs are split and
    ordered by first use across both HWDGE rings (x on SP, w on ACT).
  - Host unshard: concat 8 row-slabs, reshape to [1,512,192,192] f32.
"""

import sys

for _p in ("/opt/trn_rl_repo",):
    if _p not in sys.path:
        sys.path.insert(0, _p)

import numpy as np
import ml_dtypes

N_CORES = 8
C = 512
F = 512
H = 192
W = 192
HC = H // N_CORES          # output rows per core = 24
RP = HC // 2               # row-pairs per core = 12
PX = 2 * W                 # px per output tile = 384
CC = C // 128              # c chunks = 4
FM = F // 128              # f chunks = 4
TAPS = 9
NWARM = 14                 # PE warmup matmuls (~2.3us)
GRP = 4                    # px tiles per PSUM group (4 banks)

_CACHE = {}


def _build():
    import concourse.bacc as bacc
    import concourse.mybir as mybir
    from concourse.tile import TileContext

    BF = mybir.dt.bfloat16
    F32 = mybir.dt.float32

    nc = bacc.Bacc(trn_type="TRN2", num_devices=N_CORES)
    x_sh = nc.dram_tensor("x_sh", [128, CC, HC + 2, W + 2], BF, kind="ExternalInput")
    w_sh = nc.dram_tensor("w_sh", [128, FM, CC, TAPS, 128], BF, kind="ExternalInput")
    mb_sh = nc.dram_tensor("mb_sh", [128, RP * PX + FM], F32, kind="ExternalInput")
    y_sh = nc.dram_tensor("y_sh", [FM, 128, RP, PX], F32, kind="ExternalOutput")

    # x row-thirds, in first-use order: rows 0-9 cover group g=0 (+halo),
    # 10-17 cover g=1, 18-25 cover g=2.
    ROW_SPLITS = [(0, 10), (10, 16)]
    NG = RP // GRP

    with TileContext(nc) as tc:
        with (
            tc.tile_pool(name="const", bufs=1) as cpool,
            tc.tile_pool(name="psum", bufs=2, space="PSUM") as ppool,
            tc.tile_pool(name="outp", bufs=3) as opool,
        ):
            xt = cpool.tile([128, CC, HC + 2, W + 2], BF)
            wt = cpool.tile([128, FM, CC, TAPS, 128], BF)
            # First-use ordered loads.  x rides the SP HWDGE ring, w/mask/b
            # ride the ACT ring, so the two streams run in parallel.
            r0, nr = ROW_SPLITS[0]
            for cc in range(CC):
                nc.scalar.dma_start(out=wt[:, 0, cc], in_=w_sh[:, 0, cc])
                nc.sync.dma_start(out=xt[:, cc, r0:r0 + nr],
                                  in_=x_sh[:, cc, r0:r0 + nr])
            for r0, nr in ROW_SPLITS[1:]:
                for cc in range(CC):
                    nc.sync.dma_start(out=xt[:, cc, r0:r0 + nr],
                                      in_=x_sh[:, cc, r0:r0 + nr])
            mbt = cpool.tile([128, RP * PX + FM], F32)
            nc.scalar.dma_start(out=mbt[:], in_=mb_sh[:])
            mt = mbt[:, :RP * PX].rearrange("p (t q) -> p t q", q=PX)
            bt = mbt[:, RP * PX:]
            for fm in range(1, FM):
                nc.scalar.dma_start(out=wt[:, fm], in_=w_sh[:, fm])

            for fm in range(FM):
                for g in range(NG):
                    # one 4-bank PSUM tile holds the group's 4 output tiles
                    pt = ppool.tile([128, GRP, 512], F32, name=f"ps_{fm}_{g}",
                                    tag="ps")
                    for a in range(CC * TAPS):
                        cc, o = divmod(a, TAPS)
                        kh, kw = divmod(o, 3)
                        lhsT = wt[:, fm, cc, o]
                        for j in range(GRP):
                            t = g * GRP + j
                            rhs = xt[:, cc, 2 * t + kh:2 * t + kh + 2, kw:kw + W]
                            nc.tensor.matmul(
                                pt[:, j, :PX], lhsT, rhs,
                                start=(a == 0), stop=(a == CC * TAPS - 1),
                            )
                    last = (fm == FM - 1 and g == NG - 1)
                    if not last:
                        ot = opool.tile([128, GRP, PX], F32, name=f"ot_{fm}_{g}",
                                        tag="ot")
                        nc.vector.tensor_mul(ot[:], pt[:, :, :PX],
                                             mt[:, g * GRP:(g + 1) * GRP])
                        nc.scalar.activation(
                            ot[:], ot[:],
                            mybir.ActivationFunctionType.Identity,
                            bias=bt[:, fm:fm + 1],
                        )
                        nc.sync.dma_start(out=y_sh[fm, :, g * GRP:(g + 1) * GRP],
                                          in_=ot[:])
                    else:
                        # per-tile epilogue on the final group: keeps the
                        # exposed post-matmul chain short
                        for j in range(GRP):
                            t = g * GRP + j
                            otj = opool.tile([128, PX], F32, name=f"otl_{j}",
                                             tag="otl", bufs=4)
                            nc.vector.tensor_mul(otj[:], pt[:, j, :PX], mt[:, t])
                            nc.scalar.activation(
                                otj[:], otj[:],
                                mybir.ActivationFunctionType.Identity,
                                bias=bt[:, fm:fm + 1],
                            )
                            nc.sync.dma_start(out=y_sh[fm, :, t], in_=otj[:])

    nc.compile()
    return nc


def _pack(x, w, b, mask):
    x = np.asarray(x, dtype=np.float32)
    w = np.asarray(w, dtype=np.float32)
    b = np.asarray(b, dtype=np.float32)
    mask = np.asarray(mask)

    xp = np.zeros((C, H + 2, W + 2), dtype=np.float32)
    xp[:, 1:-1, 1:-1] = x[0]
    xp = xp.astype(ml_dtypes.bfloat16)

    # [kh,kw,c,f] -> [tap, cc, c_local, fm, f128] -> [c_local, fm, cc, tap, f128]
    wt = w.transpose(2, 3, 1, 0).reshape(TAPS, CC, 128, FM, 128)
    wt = np.ascontiguousarray(wt.transpose(2, 3, 1, 0, 4)).astype(ml_dtypes.bfloat16)

    b_re = np.ascontiguousarray(b.reshape(FM, 128).T)

    mf = mask.astype(np.float32)
    in_maps = []
    for k in range(N_CORES):
        xs = xp[:, HC * k:HC * k + HC + 2, :]                 # [512, 26, 194]
        xs = np.ascontiguousarray(
            xs.reshape(CC, 128, HC + 2, W + 2).transpose(1, 0, 2, 3))
        ms = mf[HC * k:HC * k + HC].reshape(1, RP * PX)
        mb = np.concatenate(
            [np.broadcast_to(ms, (128, RP * PX)), b_re], axis=1)
        in_maps.append({"x_sh": xs, "w_sh": wt,
                        "mb_sh": np.ascontiguousarray(mb)})
    return in_maps


def _unpack(results):
    slabs = []
    for k in range(N_CORES):
        ys = results[k]["y_sh"]                               # [4, 128, 12, 384]
        slabs.append(ys.reshape(F, HC, W))
    out = np.concatenate(slabs, axis=1)                       # [512, 192, 192]
    return out[None].astype(np.float32)


def _run(inputs, **run_kwargs):
    from concourse.bass_utils import run_bass_kernel_spmd

    if "nc" not in _CACHE:
        _CACHE["nc"] = _build()
    nc = _CACHE["nc"]
    in_maps = _pack(inputs["x"], inputs["w"], inputs["b"], inputs["mask"])
    res = run_bass_kernel_spmd(nc, in_maps, core_ids=list(range(N_CORES)), **run_kwargs)
    return _unpack(res.results), res


def kernel(**inputs):
    out, _ = _run(inputs)
    return out
